# revision 19
# baseline (speedup 1.0000x reference)
"""TRN2 Bass kernel for nn_MultiHeadAttention (B=4, S=2048, D=512, H=8).

Computation (per reference):
  v_in = LN(seq_v) ; q = seq_q@W1.T ; k = seq_k@W2.T ; v = v_in@W3.T
  scores[b,h,i,j] = k_i . q_j ; attn = softmax_j(scores) ; out = attn @ v
  out = LN(out + v_in)

Sharding (zero-communication): core c -> (batch b=c//2, i-half=c%2).
Each core computes all 8 heads for its 1024 output rows (the "i" index,
which indexes K rows), needing full q/v (all j) for its batch and the
i-half slice of k. The j axis is permuted host-side (own half first) so
one SPMD program serves all cores; softmax over j is permutation
invariant and the residual rows are j-tiles 0..7 by construction.

v2 restructure (vs the phase-serial baseline):
  - fully software-pipelined emission: LN stats, q/k/v projections and
    the attention blocks are interleaved so the first exp fires ~11us in
    (was ~75us); DMAs are spread over the sync/scalar/gpsimd queues in
    consumption order
  - ScalarE stream is pure exp (projection PSUM->SBUF copies moved to
    DVE; all rsqrt via DVE Newton iteration -> single ACT table load)
  - vaug (v values + softmax-ones column, ones first) and p (exp
    output) in bf16: halves SBUF, FWL-eligible PV weight loads; all
    accumulation stays f32
  - finalize split into per-(it,t) transpose partials (overlap later
    attention blocks) and per-it closings; softmax division via one
    reciprocal + 8 per-head tensor_scalar_muls
  - PE warmed with dummy matmuls during the DMA lead-in (HAM K=8/8)
"""

import numpy as np

B, S, D, H = 4, 2048, 512, 8
HD = D // H  # 64
EPS = 1e-5
NCORES = 8
IH = S // 2          # 1024 output rows per core
NT = S // 128        # 16 j token-tiles
ITILES = IH // 128   # 8 i-tiles
DT = D // 128        # 4 d-tiles (head pairs)
ET = D // 128        # 4 e-tiles (contraction)
NIB = IH // 512      # 2 i-blocks

_cache = {}


def _build(has_gamma: bool, has_beta: bool):
    import concourse.bacc as bacc
    import concourse.mybir as mybir
    import concourse.tile as tile
    from concourse.masks import make_identity

    f32 = mybir.dt.float32
    f32r = mybir.dt.float32r
    f16 = mybir.dt.float16
    bf16 = mybir.dt.bfloat16
    i32 = mybir.dt.int32
    Alu = mybir.AluOpType
    Act = mybir.ActivationFunctionType

    nc = bacc.Bacc(None, target_bir_lowering=False)

    sqT = nc.dram_tensor("sqT", [128, ET, S], f16, kind="ExternalInput")
    skT = nc.dram_tensor("skT", [128, ET, IH], f16, kind="ExternalInput")
    svT = nc.dram_tensor("svT", [128, ET, S], f16, kind="ExternalInput")
    sv = nc.dram_tensor("sv", [128, NT, 512], f16, kind="ExternalInput")
    w1T = nc.dram_tensor("w1T", [128, ET, D], f16, kind="ExternalInput")
    w2T = nc.dram_tensor("w2T", [128, ET, D], f16, kind="ExternalInput")
    w3gT = nc.dram_tensor("w3gT", [128, ET, D], f16, kind="ExternalInput")
    g3 = nc.dram_tensor("g3", [1, D], f32, kind="ExternalInput")
    c3v = nc.dram_tensor("c3v", [1, D], f32, kind="ExternalInput")
    gamma = nc.dram_tensor("gamma", [1, D], f32, kind="ExternalInput")
    beta = nc.dram_tensor("beta", [1, D], f32, kind="ExternalInput")
    out = nc.dram_tensor("out", [128, ITILES, D], f32, kind="ExternalOutput")

    def bcast(dram_ap):
        import concourse.bass as bass

        return bass.AP(
            tensor=dram_ap.tensor,
            offset=dram_ap.offset,
            ap=[[0, 128], [1, D]],
        )

    ts = lambda i, sz: slice(i * sz, (i + 1) * sz)

    with tile.TileContext(nc) as tc:
        with (
            tc.tile_pool(name="const", bufs=1) as const,
            tc.tile_pool(name="persist", bufs=1) as persist,
        ):
            # long-lived attention pools first (pool releases must be
            # LIFO, and these outlive the projection-phase pools)
            sps = tc.alloc_tile_pool(name="sps", bufs=2, space="PSUM")
            ops = tc.alloc_tile_pool(name="ops", bufs=1, space="PSUM")
            ppool = tc.alloc_tile_pool(name="ppool", bufs=3)

            # ---- DMA, queue sync (q/k path, in consumption order) ----
            wq_pool = tc.alloc_tile_pool(name="wqk", bufs=1)
            qs_pool = tc.alloc_tile_pool(name="qs", bufs=4)
            sk_pool = tc.alloc_tile_pool(name="sk", bufs=2)
            w1_sb = wq_pool.tile([128, ET, D], f16, tag="w1")
            nc.sync.dma_start(w1_sb, w1T[:])
            sqc = {}
            skc = {}
            sqc[0] = qs_pool.tile([128, ET, 512], f16, tag="sqc", name="sqc0")
            nc.sync.dma_start(sqc[0], sqT[:, :, 0:512])
            skc[0] = sk_pool.tile([128, ET, 512], f16, tag="skc", name="skc0")
            _anchor = nc.sync.dma_start(skc[0], skT[:, :, 0:512])
            # (sqc1..3, skc1 issued below, after the first compute units)

            def held(bi):
                # bulk transfers yield HBM to the critical prefix: they
                # wait for skc0 (last prefix transfer) to complete
                tile.add_dep_helper(
                    bi.ins, _anchor.ins, sync=True, reason="dma priority"
                )
                return bi

            # ---- DMA, queue scalar (w2 only; before any exp) ----
            w2_sb = wq_pool.tile([128, ET, D], f16, tag="w2")
            nc.scalar.dma_start(w2_sb, w2T[:])

            # ---- DMA, queues scalar/gpsimd (v path + consts) ----
            g3b = const.tile([128, D], f32, tag="g3b")
            _g3b_dma = nc.gpsimd.dma_start(g3b, bcast(g3[:]))
            if has_gamma:
                gammab = const.tile([128, D], f32, tag="gammab")
                nc.gpsimd.dma_start(gammab, bcast(gamma[:]))
            if has_beta:
                betab = const.tile([128, D], f32, tag="betab")
                nc.gpsimd.dma_start(betab, bcast(beta[:]))
                c3b = const.tile([128, D], f32, tag="c3b")
                nc.gpsimd.dma_start(c3b, bcast(c3v[:]))
            w3_pool = tc.alloc_tile_pool(name="w3p", bufs=1)
            w3_sb = w3_pool.tile([128, ET, D], f16, tag="w3")
            held(nc.scalar.dma_start(w3_sb, w3gT[:]))

            tile.add_dep_helper(
                _g3b_dma.ins, _anchor.ins, sync=True, reason="dma priority"
            )

            # persistent intermediates
            qT_sb = persist.tile([128, DT, S], f16, tag="qT")
            kT_sb = persist.tile([128, DT, IH], f16, tag="kT")
            vaug = persist.tile([128, NT, H, 65], bf16, tag="vaug")
            outT_e = persist.tile([65, DT, IH], f32, tag="outTe")
            outT_o = persist.tile([65, DT, IH], f32, tag="outTo")
            vinres = persist.tile([128, ITILES, 512], f16, tag="vinres")
            mu16 = persist.tile([128, NT], f32, tag="mu16")
            ve16 = persist.tile([128, NT], f32, tag="ve16")
            rstd16 = persist.tile([128, NT], f32, tag="rstd16")
            mr16 = persist.tile([128, NT], f32, tag="mr16")

            # sv own-half lands directly in vinres (LN applied in-place
            # later); other-half tiles stream through the stat pool
            nc.gpsimd.dma_start(vinres[:, 0:4, :], sv[:, 0:4, :])
            held(nc.gpsimd.dma_start(vinres[:, 4:8, :], sv[:, 4:8, :]))
            svc = {}
            st_pool = tc.alloc_tile_pool(name="stat", bufs=2)
            vs_pool = tc.alloc_tile_pool(name="vs", bufs=3)
            for c in range(8):
                svc[c] = vs_pool.tile(
                    [128, ET, 256], f16, tag="svc", name=f"svc{c}"
                )
                eng = nc.scalar if c < 4 else nc.gpsimd
                held(eng.dma_start(svc[c], svT[:, :, ts(c, 256)]))
            # LN-stats input for the other j-half
            xoh = {}
            for c in range(4):
                xoh[c] = st_pool.tile(
                    [128, 2, 512], f16, tag="x", name=f"xoh{c}", bufs=2
                )
                held(nc.gpsimd.dma_start(xoh[c], sv[:, ts(c + 4, 2), :]))

            onesc = const.tile([128, NT * H], bf16, tag="onesc")
            nc.vector.memset(onesc, 1.0)
            nc.vector.tensor_copy(
                vaug[:, :, :, 0],
                onesc.rearrange("p (a b) -> p a b", a=NT),
            )
            ident = const.tile([128, 128], f32, tag="ident")
            make_identity(nc, ident)

            pp_pool = tc.alloc_tile_pool(name="pp", bufs=2, space="PSUM")

            # ---- helper units ----
            def newton_rsqrt(dst, src, n, tmp_pool):
                # dst = 1/sqrt(src) elementwise on [128, n] (DVE only)
                nc.vector.tensor_scalar(
                    out=dst.bitcast(i32),
                    in0=src.bitcast(i32),
                    scalar1=1,
                    scalar2=None,
                    op0=Alu.logical_shift_right,
                )
                nc.vector.tensor_scalar(
                    out=dst.bitcast(i32),
                    in0=dst.bitcast(i32),
                    scalar1=-1,
                    scalar2=0x5F3759DF,
                    op0=Alu.mult,
                    op1=Alu.add,
                )
                tmp1 = tmp_pool.tile([128, NT], f32, tag="nwt")
                t1 = tmp1[:, 0:n]
                for _ in range(2):
                    nc.vector.tensor_mul(t1, dst, dst)
                    nc.vector.tensor_mul(t1, t1, src)
                    nc.vector.tensor_scalar(
                        out=t1,
                        in0=t1,
                        scalar1=-0.5,
                        scalar2=1.5,
                        op0=Alu.mult,
                        op1=Alu.add,
                    )
                    nc.vector.tensor_mul(dst, dst, t1)

            def stats_unit(jt, x):
                # LN stats of token tile jt from x = [128, 512]
                st = st_pool.tile([128, 6], f32, tag="st")
                nc.vector.bn_stats(st, x)
                mv = st_pool.tile([128, 2], f32, tag="mv")
                nc.vector.bn_aggr(mv, st)
                nc.vector.tensor_copy(mu16[:, jt : jt + 1], mv[:, 0:1])
                nc.vector.tensor_scalar_add(
                    ve16[:, jt : jt + 1], mv[:, 1:2], EPS
                )

            def rstd_batch(lo, hi):
                newton_rsqrt(
                    rstd16[:, lo:hi], ve16[:, lo:hi], hi - lo, st_pool
                )
                nc.vector.tensor_mul(
                    mr16[:, lo:hi], mu16[:, lo:hi], rstd16[:, lo:hi]
                )

            def qproj(jc, t):
                ps = pp_pool.tile([128, 512], f32, tag="pp")
                for e in range(ET):
                    nc.tensor.matmul(
                        ps,
                        w1_sb[:, e, ts(t, 128)],
                        sqc[jc][:, e, :],
                        start=(e == 0),
                        stop=(e == ET - 1),
                    )
                nc.vector.tensor_copy(qT_sb[:, t, ts(jc, 512)], ps)

            def kproj(ic, t):
                ps = pp_pool.tile([128, 512], f32, tag="pp")
                for e in range(ET):
                    nc.tensor.matmul(
                        ps,
                        w2_sb[:, e, ts(t, 128)],
                        skc[ic][:, e, :],
                        start=(e == 0),
                        stop=(e == ET - 1),
                    )
                nc.vector.tensor_copy(kT_sb[:, t, ts(ic, 512)], ps)

            def vproj(jt):
                # v row jt: vaug[:, jt, h, 1:65] = rstd*(sv@W3g.T) - mr*g3 (+c3)
                ps = pp_pool.tile([128, 512], f32, tag="pp")
                c, r = divmod(jt, 2)
                for e in range(ET):
                    nc.tensor.matmul(
                        ps,
                        svc[c][:, e, ts(r, 128)],
                        w3_sb[:, e, :],
                        start=(e == 0),
                        stop=(e == ET - 1),
                    )
                tA = vs_pool.tile([128, 512], bf16, tag="tA", bufs=2)
                nc.vector.tensor_scalar_mul(tA, g3b, mr16[:, jt : jt + 1])
                tB = vs_pool.tile([128, 512], bf16, tag="tB", bufs=2)
                nc.vector.tensor_scalar_mul(tB, ps, rstd16[:, jt : jt + 1])
                vdst = vaug[:, jt, :, 1:65]
                nc.vector.tensor_tensor(
                    out=vdst,
                    in0=tB.rearrange("p (h d) -> p h d", h=H),
                    in1=tA.rearrange("p (h d) -> p h d", h=H),
                    op=Alu.subtract,
                )
                if has_beta:
                    nc.gpsimd.tensor_add(
                        vdst,
                        vdst,
                        c3b.rearrange("p (h d) -> p h d", h=H),
                    )

            def vinres_unit(jt):
                # in-place LN of own-half sv tile (residual v_in)
                nc.vector.tensor_scalar(
                    out=vinres[:, jt, :],
                    in0=vinres[:, jt, :],
                    scalar1=mu16[:, jt : jt + 1],
                    scalar2=rstd16[:, jt : jt + 1],
                    op0=Alu.subtract,
                    op1=Alu.mult,
                )
                if has_gamma:
                    nc.vector.tensor_mul(
                        vinres[:, jt, :], vinres[:, jt, :], gammab
                    )
                if has_beta:
                    nc.gpsimd.tensor_add(
                        vinres[:, jt, :], vinres[:, jt, :], betab
                    )

            # ---- phase A: minimal prefix during the DMA lead-in ----
            for jt in range(4):
                stats_unit(jt, vinres[:, jt, :])
            qproj(0, 0)
            kproj(0, 0)
            rstd_batch(0, 4)
            # remaining q-path DMAs (sync queue, in need order)
            for jc in range(1, 4):
                sqc[jc] = qs_pool.tile(
                    [128, ET, 512], f16, tag="sqc", name=f"sqc{jc}"
                )
                nc.sync.dma_start(sqc[jc], sqT[:, :, ts(jc, 512)])
            skc[1] = sk_pool.tile([128, ET, 512], f16, tag="skc", name="skc1")
            nc.sync.dma_start(skc[1], skT[:, :, 512:1024])
            for jt in range(4, 8):
                stats_unit(jt, vinres[:, jt, :])
            rstd_batch(4, 8)
            for jt in range(4):
                vproj(jt)
            for jt in range(8, 16):
                stats_unit(jt, xoh[(jt - 8) // 2][:, (jt - 8) % 2, :])
            rstd_batch(8, 16)

            # ---- attention + overlapped projections/finalize ----
            ydn_tiles = {}
            fps_box = {}

            if True:

                def attn_block(t, ib, interleave=None):
                    o_e = ops.tile([65, 512], f32, tag="oe")
                    o_o = ops.tile([65, 512], f32, tag="oo")

                    def pv(jt, p):
                        nc.tensor.matmul(
                            o_e,
                            vaug[:, jt, 2 * t, :],
                            p[:, 0:512],
                            start=(jt == 0),
                            stop=(jt == NT - 1),
                        )
                        nc.tensor.matmul(
                            o_o,
                            vaug[:, jt, 2 * t + 1, :],
                            p[:, 512:1024],
                            start=(jt == 0),
                            stop=(jt == NT - 1),
                        )

                    prev = None
                    for jt in range(NT):
                        if interleave is not None:
                            interleave(jt)
                        s = sps.tile([128, 1024], f32, tag="s")
                        nc.tensor.matmul(
                            s[:, 0:512],
                            qT_sb[0:64, t, ts(jt, 128)],
                            kT_sb[0:64, t, ts(ib, 512)],
                            start=True,
                            stop=True,
                        )
                        nc.tensor.matmul(
                            s[:, 512:1024],
                            qT_sb[64:128, t, ts(jt, 128)],
                            kT_sb[64:128, t, ts(ib, 512)],
                            start=True,
                            stop=True,
                        )
                        p = ppool.tile([128, 1024], bf16, tag="p")
                        nc.scalar.activation(p, s, Act.Exp)
                        if prev is not None:
                            pv(*prev)
                        prev = (jt, p)
                    pv(*prev)
                    nc.scalar.copy(outT_e[:, t, ts(ib, 512)], o_e)
                    nc.vector.tensor_copy(outT_o[:, t, ts(ib, 512)], o_o)

                y_tiles = {}
                mv_tiles = {}

                def partial(it, t):
                    # PE-transpose outT head-pair t for i-tile it into ydn,
                    # then divide by the softmax denominators right away
                    ydnp, fin, fsc, fps = fps_box["pools"]
                    if it not in ydn_tiles:
                        ydn_tiles[it] = ydnp.tile(
                            [128, 8, 65], f32, tag="ydn", name=f"ydn{it}"
                        )
                        y_tiles[it] = fin.tile(
                            [128, 512], f32, tag="y", name=f"y{it}", bufs=8
                        )
                    ydn = ydn_tiles[it]
                    y = y_tiles[it]
                    for so, srcT in ((0, outT_e), (1, outT_o)):
                        tp = fps.tile([128, 65], f32, tag="tp")
                        nc.tensor.transpose(
                            tp,
                            srcT[0:65, t, ts(it, 128)],
                            ident[0:65, 0:65],
                        )
                        nc.vector.tensor_copy(ydn[:, 2 * t + so, :], tp)
                    rc2 = fsc.tile([128, 2], f32, tag="rc2")
                    nc.vector.reciprocal(rc2, ydn[:, 2 * t : 2 * t + 2, 0])
                    for so in range(2):
                        k = 2 * t + so
                        nc.vector.tensor_scalar_mul(
                            y[:, ts(k, 64)],
                            ydn[:, k, 1:65],
                            rc2[:, so : so + 1],
                        )

                def closing_a(it, ve):
                    ydnp, fin, fsc, fps = fps_box["pools"]
                    ydn_tiles.pop(it)
                    y = y_tiles[it]
                    nc.vector.tensor_add(y, y, vinres[:, it, :])
                    st = fsc.tile([128, 6], f32, tag="fst")
                    nc.vector.bn_stats(st, y)
                    mv = fsc.tile([128, 2], f32, tag="fmv", bufs=8)
                    nc.vector.bn_aggr(mv, st)
                    mv_tiles[it] = mv
                    nc.vector.tensor_scalar_add(ve, mv[:, 1:2], EPS)

                def closing_b(it, rstd2):
                    y = y_tiles.pop(it)
                    mv = mv_tiles.pop(it)
                    nc.vector.tensor_scalar(
                        out=y,
                        in0=y,
                        scalar1=mv[:, 0:1],
                        scalar2=rstd2,
                        op0=Alu.subtract,
                        op1=Alu.mult,
                    )
                    if has_gamma:
                        nc.vector.tensor_mul(y, y, gammab)
                    if has_beta:
                        nc.gpsimd.tensor_add(y, y, betab)
                    nc.sync.dma_start(out[:, it, :], y)

                def closing(it):
                    ydnp, fin, fsc, fps = fps_box["pools"]
                    ve = fsc.tile([128, 1], f32, tag="fve")
                    closing_a(it, ve)
                    rstd2 = fsc.tile([128, 1], f32, tag="frs")
                    newton_rsqrt(rstd2, ve, 1, fsc)
                    closing_b(it, rstd2)

                def closing_batch(its):
                    ydnp, fin, fsc, fps = fps_box["pools"]
                    ve4 = fsc.tile([128, 4], f32, tag="fve4")
                    for j, it in enumerate(its):
                        closing_a(it, ve4[:, j : j + 1])
                    rstd4 = fsc.tile([128, 4], f32, tag="frs4")
                    newton_rsqrt(rstd4, ve4, 4, fsc)
                    for j, it in enumerate(its):
                        closing_b(it, rstd4[:, j : j + 1])

                # interleave tables: JIT projections + v-proj stream;
                # vinres rides later blocks' spare DVE slots
                def il_b00(jt):
                    if jt < 12:
                        vproj(jt + 4)
                    if jt in (1, 5, 9):
                        qproj(jt // 4 + 1, 0)
                    elif jt == 13:
                        qproj(0, 1)
                    elif jt == 14:
                        kproj(0, 1)
                    elif jt == 15:
                        vinres_unit(0)

                def il_b10(jt):
                    if jt in (1, 5, 9):
                        qproj((jt + 3) // 4, 1)
                    elif jt == 2:
                        qproj(0, 2)
                    elif jt == 6:
                        kproj(0, 2)
                    elif jt == 10:
                        qproj(0, 3)
                    elif jt == 12:
                        kproj(0, 3)
                    elif jt in (3, 7, 11, 13):
                        vinres_unit(1 + (jt - 3) // 4 + (1 if jt == 13 else 0))

                def il_b20(jt):
                    if jt in (1, 3, 5):
                        qproj((jt + 1) // 2, 2)
                    elif jt in (7, 9, 11):
                        qproj((jt - 5) // 2, 3)
                    elif jt >= 12:
                        kproj(1, jt - 12)
                    elif jt in (2, 4, 6):
                        vinres_unit(5 + (jt - 2) // 2)

                attn_block(0, 0, interleave=il_b00)
                attn_block(1, 0, interleave=il_b10)
                attn_block(2, 0, interleave=il_b20)
                # all projections emitted; free their SBUF/PSUM for the
                # finalize pools
                pp_pool.release()
                vs_pool.release()
                st_pool.release()
                w3_pool.release()
                sk_pool.release()
                qs_pool.release()
                wq_pool.release()
                fps_box["pools"] = (
                    tc.alloc_tile_pool(name="ydnp", bufs=4),
                    tc.alloc_tile_pool(name="fin", bufs=4),
                    tc.alloc_tile_pool(name="fsc", bufs=4),
                    tc.alloc_tile_pool(name="fps", bufs=2, space="PSUM"),
                )
                for t2 in range(3):
                    for it in range(4):
                        partial(it, t2)
                attn_block(3, 0)
                for it in range(4):
                    partial(it, 3)

                def make_il_ib1(t):
                    def il(jt):
                        if t > 0 and jt < 4:
                            partial(4 + jt, t - 1)
                        if jt == 6 and t > 0:
                            closing(t - 1)
                        elif jt == 10 and t == 3:
                            closing(3)

                    return il

                for t in range(DT):
                    attn_block(t, 1, interleave=make_il_ib1(t))
                for it in range(4, 8):
                    partial(it, 3)
                closing_batch(range(4, 8))
                for pool in reversed(fps_box["pools"]):
                    pool.release()
            ppool.release()
            ops.release()
            sps.release()

    nc.compile()
    return nc


def _to_tiles_T(x):
    # [N, 512] -> [128, 4, N] f16 : out[p, t, n] = x[n, 128*t + p]
    n = x.shape[0]
    return np.ascontiguousarray(
        x.T.reshape(ET, 128, n).transpose(1, 0, 2), dtype=np.float16
    )


def _w_tiles(w):
    # [512, 512] (e, d) -> [128, 4, 512] f16 : out[p, t, d] = w[128*t + p, d]
    return np.ascontiguousarray(
        w.reshape(ET, 128, D).transpose(1, 0, 2), dtype=np.float16
    )


def kernel(seq_k, seq_q, seq_v, W1, W2, W3, gamma, beta, _trace=False):
    seq_k = np.asarray(seq_k, dtype=np.float32)
    seq_q = np.asarray(seq_q, dtype=np.float32)
    seq_v = np.asarray(seq_v, dtype=np.float32)
    W1 = np.asarray(W1, dtype=np.float32)
    W2 = np.asarray(W2, dtype=np.float32)
    W3 = np.asarray(W3, dtype=np.float32)
    gamma = np.asarray(gamma, dtype=np.float32)
    beta = np.asarray(beta, dtype=np.float32)

    has_gamma = bool(np.any(gamma != 1.0))
    has_beta = bool(np.any(beta != 0.0))

    key = (has_gamma, has_beta)
    if key not in _cache:
        _cache[key] = _build(has_gamma, has_beta)
    nc = _cache[key]

    from concourse import bass_utils

    W3g = W3 * gamma[None, :]  # W3g[d, e] = W3[d, e] * gamma[e]
    g3v = np.ascontiguousarray((W3 @ gamma)[None, :], dtype=np.float32)
    c3vv = np.ascontiguousarray((W3 @ beta)[None, :], dtype=np.float32)
    w1t = _w_tiles(np.ascontiguousarray(W1.T))
    w2t = _w_tiles(np.ascontiguousarray(W2.T))
    w3t = _w_tiles(np.ascontiguousarray(W3g.T))
    gam = np.ascontiguousarray(gamma[None, :], dtype=np.float32)
    bet = np.ascontiguousarray(beta[None, :], dtype=np.float32)

    in_maps = []
    for c in range(NCORES):
        b, half = divmod(c, 2)
        lo, hi = half * IH, half * IH + IH
        perm = np.r_[lo:hi, 0:lo, hi:S]
        sq = seq_q[b][perm]
        svp = seq_v[b][perm]
        sk = seq_k[b, lo:hi]
        in_maps.append(
            {
                "sqT": _to_tiles_T(sq),
                "skT": _to_tiles_T(sk),
                "svT": _to_tiles_T(svp),
                "sv": np.ascontiguousarray(
                    svp.reshape(NT, 128, 512).transpose(1, 0, 2),
                    dtype=np.float16,
                ),
                "w1T": w1t,
                "w2T": w2t,
                "w3gT": w3t,
                "g3": g3v,
                "c3v": c3vv,
                "gamma": gam,
                "beta": bet,
            }
        )

    res = bass_utils.run_bass_kernel_spmd(
        nc, in_maps, core_ids=list(range(NCORES)), trace=_trace
    )
    global _last_run
    _last_run = res

    full = np.empty((B, S, D), dtype=np.float32)
    for c in range(NCORES):
        b, half = divmod(c, 2)
        o = res.results[c]["out"]  # [128, 8, 512]
        full[b, half * IH : (half + 1) * IH] = o.transpose(1, 0, 2).reshape(
            IH, D
        )
    return full


_last_run = None


# revision 22
# speedup vs baseline: 1.0379x; 1.0379x over previous
"""TRN2 Bass kernel for nn_MultiHeadAttention (B=4, S=2048, D=512, H=8).

Computation (per reference):
  v_in = LN(seq_v) ; q = seq_q@W1.T ; k = seq_k@W2.T ; v = v_in@W3.T
  scores[b,h,i,j] = k_i . q_j ; attn = softmax_j(scores) ; out = attn @ v
  out = LN(out + v_in)

Sharding (zero-communication): core c -> (batch b=c//2, i-half=c%2).
Each core computes all 8 heads for its 1024 output rows (the "i" index,
which indexes K rows), needing full q/v (all j) for its batch and the
i-half slice of k. The j axis is permuted host-side (own half first) so
one SPMD program serves all cores; softmax over j is permutation
invariant and the residual rows are j-tiles 0..7 by construction.

v2 restructure (vs the phase-serial baseline):
  - fully software-pipelined emission: LN stats, q/k/v projections and
    the attention blocks are interleaved so the first exp fires ~11us in
    (was ~75us); DMAs are spread over the sync/scalar/gpsimd queues in
    consumption order
  - ScalarE stream is pure exp (projection PSUM->SBUF copies moved to
    DVE; all rsqrt via DVE Newton iteration -> single ACT table load)
  - vaug (v values + softmax-ones column, ones first) and p (exp
    output) in bf16: halves SBUF, FWL-eligible PV weight loads; all
    accumulation stays f32
  - finalize split into per-(it,t) transpose partials (overlap later
    attention blocks) and per-it closings; softmax division via one
    reciprocal + 8 per-head tensor_scalar_muls
  - PE warmed with dummy matmuls during the DMA lead-in (HAM K=8/8)
"""

import numpy as np

B, S, D, H = 4, 2048, 512, 8
HD = D // H  # 64
EPS = 1e-5
NCORES = 8
IH = S // 2          # 1024 output rows per core
NT = S // 128        # 16 j token-tiles
ITILES = IH // 128   # 8 i-tiles
DT = D // 128        # 4 d-tiles (head pairs)
ET = D // 128        # 4 e-tiles (contraction)
NIB = IH // 512      # 2 i-blocks

_cache = {}


def _build(has_gamma: bool, has_beta: bool):
    import concourse.bacc as bacc
    import concourse.mybir as mybir
    import concourse.tile as tile
    from concourse.masks import make_identity

    f32 = mybir.dt.float32
    f32r = mybir.dt.float32r
    f16 = mybir.dt.float16
    bf16 = mybir.dt.bfloat16
    i32 = mybir.dt.int32
    Alu = mybir.AluOpType
    Act = mybir.ActivationFunctionType

    nc = bacc.Bacc(None, target_bir_lowering=False)

    sqT = nc.dram_tensor("sqT", [128, ET, S], f16, kind="ExternalInput")
    skT = nc.dram_tensor("skT", [128, ET, IH], f16, kind="ExternalInput")
    svT = nc.dram_tensor("svT", [128, ET, S], f16, kind="ExternalInput")
    sv = nc.dram_tensor("sv", [128, NT, 512], f16, kind="ExternalInput")
    w1T = nc.dram_tensor("w1T", [128, ET, D], f16, kind="ExternalInput")
    w2T = nc.dram_tensor("w2T", [128, ET, D], f16, kind="ExternalInput")
    w3gT = nc.dram_tensor("w3gT", [128, ET, D], f16, kind="ExternalInput")
    g3 = nc.dram_tensor("g3", [1, D], f32, kind="ExternalInput")
    c3v = nc.dram_tensor("c3v", [1, D], f32, kind="ExternalInput")
    gamma = nc.dram_tensor("gamma", [1, D], f32, kind="ExternalInput")
    beta = nc.dram_tensor("beta", [1, D], f32, kind="ExternalInput")
    out = nc.dram_tensor("out", [128, ITILES, D], f32, kind="ExternalOutput")

    def bcast(dram_ap):
        import concourse.bass as bass

        return bass.AP(
            tensor=dram_ap.tensor,
            offset=dram_ap.offset,
            ap=[[0, 128], [1, D]],
        )

    ts = lambda i, sz: slice(i * sz, (i + 1) * sz)

    with tile.TileContext(nc) as tc:
        with (
            tc.tile_pool(name="const", bufs=1) as const,
            tc.tile_pool(name="persist", bufs=1) as persist,
        ):
            # long-lived attention pools first (pool releases must be
            # LIFO, and these outlive the projection-phase pools)
            sps = tc.alloc_tile_pool(name="sps", bufs=2, space="PSUM")
            ops = tc.alloc_tile_pool(name="ops", bufs=1, space="PSUM")
            ppool = tc.alloc_tile_pool(name="ppool", bufs=3)

            # ---- DMA, queue sync (q/k path, in consumption order) ----
            wq_pool = tc.alloc_tile_pool(name="wqk", bufs=1)
            qs_pool = tc.alloc_tile_pool(name="qs", bufs=4)
            sk_pool = tc.alloc_tile_pool(name="sk", bufs=2)
            w1_sb = wq_pool.tile([128, ET, D], f16, tag="w1")
            nc.sync.dma_start(w1_sb, w1T[:])
            sqc = {}
            skc = {}
            sqc[0] = qs_pool.tile([128, ET, 512], f16, tag="sqc", name="sqc0")
            nc.sync.dma_start(sqc[0], sqT[:, :, 0:512])
            skc[0] = sk_pool.tile([128, ET, 512], f16, tag="skc", name="skc0")
            nc.sync.dma_start(skc[0], skT[:, :, 0:512])
            # (sqc1..3, skc1 issued below, after the first compute units)
            _anchors = {}
            _held = []

            def held(bi, key="v"):
                # bulk transfers yield HBM to the critical prefix (dep
                # applied immediately once the anchor transfer is known)
                if key in _anchors:
                    tile.add_dep_helper(
                        bi.ins,
                        _anchors[key].ins,
                        sync=True,
                        reason="dma prio",
                    )
                else:
                    _held.append((bi, key))
                return bi

            # ---- DMA, queue scalar (w2 only; before any exp) ----
            w2_sb = wq_pool.tile([128, ET, D], f16, tag="w2")
            nc.scalar.dma_start(w2_sb, w2T[:])

            # ---- DMA, queues scalar/gpsimd (v path + consts) ----
            g3b = const.tile([128, D], f32, tag="g3b")
            _g3b_dma = nc.gpsimd.dma_start(g3b, bcast(g3[:]))
            if has_gamma:
                gammab = const.tile([128, D], f32, tag="gammab")
                nc.gpsimd.dma_start(gammab, bcast(gamma[:]))
            if has_beta:
                betab = const.tile([128, D], f32, tag="betab")
                nc.gpsimd.dma_start(betab, bcast(beta[:]))
                c3b = const.tile([128, D], f32, tag="c3b")
                nc.gpsimd.dma_start(c3b, bcast(c3v[:]))
            w3_pool = tc.alloc_tile_pool(name="w3p", bufs=1)
            w3_sb = w3_pool.tile([128, ET, D], f16, tag="w3")
            held(nc.scalar.dma_start(w3_sb, w3gT[:]))



            # persistent intermediates
            qT_sb = persist.tile([128, DT, S], f16, tag="qT")
            kT_sb = persist.tile([128, DT, IH], f16, tag="kT")
            vaug = persist.tile([128, NT, H, 65], bf16, tag="vaug")
            outT_e = persist.tile([65, DT, IH], f32, tag="outTe")
            outT_o = persist.tile([65, DT, IH], f32, tag="outTo")
            vinres = persist.tile([128, ITILES, 512], f16, tag="vinres")
            mu16 = persist.tile([128, NT], f32, tag="mu16")
            ve16 = persist.tile([128, NT], f32, tag="ve16")
            rstd16 = persist.tile([128, NT], f32, tag="rstd16")
            mr16 = persist.tile([128, NT], f32, tag="mr16")

            # sv own-half lands directly in vinres (LN applied in-place
            # later); other-half tiles stream through the stat pool
            _anchors["v"] = nc.gpsimd.dma_start(
                vinres[:, 0:4, :], sv[:, 0:4, :]
            )
            held(nc.gpsimd.dma_start(vinres[:, 4:8, :], sv[:, 4:8, :]))
            svc = {}
            st_pool = tc.alloc_tile_pool(name="stat", bufs=2)
            vs_pool = tc.alloc_tile_pool(name="vs", bufs=3)
            for c in range(8):
                svc[c] = vs_pool.tile(
                    [128, ET, 256], f16, tag="svc", name=f"svc{c}"
                )
                eng = nc.scalar if c < 4 else nc.gpsimd
                h = held(eng.dma_start(svc[c], svT[:, :, ts(c, 256)]))
                if c == 1:
                    _anchors["q"] = h
            # LN-stats input for the other j-half
            xoh = {}
            for c in range(4):
                xoh[c] = st_pool.tile(
                    [128, 2, 512], f16, tag="x", name=f"xoh{c}", bufs=2
                )
                held(nc.gpsimd.dma_start(xoh[c], sv[:, ts(c + 4, 2), :]))

            held(_g3b_dma)
            for _bi, _key in _held:
                tile.add_dep_helper(
                    _bi.ins, _anchors[_key].ins, sync=True, reason="dma prio"
                )
            onesc = const.tile([128, NT * H], bf16, tag="onesc")
            nc.vector.memset(onesc, 1.0)
            nc.vector.tensor_copy(
                vaug[:, :, :, 0],
                onesc.rearrange("p (a b) -> p a b", a=NT),
            )
            ident = const.tile([128, 128], f32, tag="ident")
            make_identity(nc, ident)

            pp_pool = tc.alloc_tile_pool(name="pp", bufs=2, space="PSUM")

            # ---- helper units ----
            def newton_rsqrt(dst, src, n, tmp_pool):
                # dst = 1/sqrt(src) elementwise on [128, n] (DVE only)
                nc.vector.tensor_scalar(
                    out=dst.bitcast(i32),
                    in0=src.bitcast(i32),
                    scalar1=1,
                    scalar2=None,
                    op0=Alu.logical_shift_right,
                )
                nc.vector.tensor_scalar(
                    out=dst.bitcast(i32),
                    in0=dst.bitcast(i32),
                    scalar1=-1,
                    scalar2=0x5F3759DF,
                    op0=Alu.mult,
                    op1=Alu.add,
                )
                tmp1 = tmp_pool.tile([128, NT], f32, tag="nwt")
                t1 = tmp1[:, 0:n]
                for _ in range(2):
                    nc.vector.tensor_mul(t1, dst, dst)
                    nc.vector.tensor_mul(t1, t1, src)
                    nc.vector.tensor_scalar(
                        out=t1,
                        in0=t1,
                        scalar1=-0.5,
                        scalar2=1.5,
                        op0=Alu.mult,
                        op1=Alu.add,
                    )
                    nc.vector.tensor_mul(dst, dst, t1)

            def stats_unit(jt, x):
                # LN stats of token tile jt from x = [128, 512]
                st = st_pool.tile([128, 6], f32, tag="st")
                nc.vector.bn_stats(st, x)
                mv = st_pool.tile([128, 2], f32, tag="mv")
                nc.vector.bn_aggr(mv, st)
                nc.vector.tensor_copy(mu16[:, jt : jt + 1], mv[:, 0:1])
                nc.vector.tensor_scalar_add(
                    ve16[:, jt : jt + 1], mv[:, 1:2], EPS
                )

            def rstd_batch(lo, hi):
                newton_rsqrt(
                    rstd16[:, lo:hi], ve16[:, lo:hi], hi - lo, st_pool
                )
                nc.vector.tensor_mul(
                    mr16[:, lo:hi], mu16[:, lo:hi], rstd16[:, lo:hi]
                )

            def qproj(jc, t):
                ps = pp_pool.tile([128, 512], f32, tag="pp")
                for e in range(ET):
                    nc.tensor.matmul(
                        ps,
                        w1_sb[:, e, ts(t, 128)],
                        sqc[jc][:, e, :],
                        start=(e == 0),
                        stop=(e == ET - 1),
                    )
                nc.vector.tensor_copy(qT_sb[:, t, ts(jc, 512)], ps)

            def kproj(ic, t):
                ps = pp_pool.tile([128, 512], f32, tag="pp")
                for e in range(ET):
                    nc.tensor.matmul(
                        ps,
                        w2_sb[:, e, ts(t, 128)],
                        skc[ic][:, e, :],
                        start=(e == 0),
                        stop=(e == ET - 1),
                    )
                nc.vector.tensor_copy(kT_sb[:, t, ts(ic, 512)], ps)

            def vproj(jt):
                # v row jt: vaug[:, jt, h, 1:65] = rstd*(sv@W3g.T) - mr*g3 (+c3)
                ps = pp_pool.tile([128, 512], f32, tag="pp")
                c, r = divmod(jt, 2)
                for e in range(ET):
                    nc.tensor.matmul(
                        ps,
                        svc[c][:, e, ts(r, 128)],
                        w3_sb[:, e, :],
                        start=(e == 0),
                        stop=(e == ET - 1),
                    )
                tA = vs_pool.tile([128, 512], bf16, tag="tA", bufs=2)
                nc.vector.tensor_scalar_mul(tA, g3b, mr16[:, jt : jt + 1])
                tB = vs_pool.tile([128, 512], bf16, tag="tB", bufs=2)
                nc.vector.tensor_scalar_mul(tB, ps, rstd16[:, jt : jt + 1])
                vdst = vaug[:, jt, :, 1:65]
                nc.vector.tensor_tensor(
                    out=vdst,
                    in0=tB.rearrange("p (h d) -> p h d", h=H),
                    in1=tA.rearrange("p (h d) -> p h d", h=H),
                    op=Alu.subtract,
                )
                if has_beta:
                    nc.gpsimd.tensor_add(
                        vdst,
                        vdst,
                        c3b.rearrange("p (h d) -> p h d", h=H),
                    )

            def vinres_unit(jt):
                # in-place LN of own-half sv tile (residual v_in)
                nc.vector.tensor_scalar(
                    out=vinres[:, jt, :],
                    in0=vinres[:, jt, :],
                    scalar1=mu16[:, jt : jt + 1],
                    scalar2=rstd16[:, jt : jt + 1],
                    op0=Alu.subtract,
                    op1=Alu.mult,
                )
                if has_gamma:
                    nc.vector.tensor_mul(
                        vinres[:, jt, :], vinres[:, jt, :], gammab
                    )
                if has_beta:
                    nc.gpsimd.tensor_add(
                        vinres[:, jt, :], vinres[:, jt, :], betab
                    )

            # ---- phase A: minimal prefix during the DMA lead-in ----
            for jt in range(4):
                stats_unit(jt, vinres[:, jt, :])
            qproj(0, 0)
            kproj(0, 0)
            rstd_batch(0, 4)
            # remaining q-path DMAs (sync queue, in need order)
            for jc in range(1, 4):
                sqc[jc] = qs_pool.tile(
                    [128, ET, 512], f16, tag="sqc", name=f"sqc{jc}"
                )
                h = nc.sync.dma_start(sqc[jc], sqT[:, :, ts(jc, 512)])
                if jc >= 2:
                    held(h, "q")
            skc[1] = sk_pool.tile([128, ET, 512], f16, tag="skc", name="skc1")
            held(nc.sync.dma_start(skc[1], skT[:, :, 512:1024]), "q")
            for jt in range(4, 8):
                stats_unit(jt, vinres[:, jt, :])
            rstd_batch(4, 8)
            for jt in range(4):
                vproj(jt)
            for jt in range(8, 16):
                stats_unit(jt, xoh[(jt - 8) // 2][:, (jt - 8) % 2, :])
            rstd_batch(8, 16)

            # ---- attention + overlapped projections/finalize ----
            ydn_tiles = {}
            fps_box = {}

            if True:

                def attn_block(t, ib, interleave=None):
                    o_e = ops.tile([65, 512], f32, tag="oe")
                    o_o = ops.tile([65, 512], f32, tag="oo")

                    def pv(jt, p):
                        nc.tensor.matmul(
                            o_e,
                            vaug[:, jt, 2 * t, :],
                            p[:, 0:512],
                            start=(jt == 0),
                            stop=(jt == NT - 1),
                        )
                        nc.tensor.matmul(
                            o_o,
                            vaug[:, jt, 2 * t + 1, :],
                            p[:, 512:1024],
                            start=(jt == 0),
                            stop=(jt == NT - 1),
                        )

                    prev = None
                    for jt in range(NT):
                        if interleave is not None:
                            interleave(jt)
                        s = sps.tile([128, 1024], f32, tag="s")
                        nc.tensor.matmul(
                            s[:, 0:512],
                            qT_sb[0:64, t, ts(jt, 128)],
                            kT_sb[0:64, t, ts(ib, 512)],
                            start=True,
                            stop=True,
                        )
                        nc.tensor.matmul(
                            s[:, 512:1024],
                            qT_sb[64:128, t, ts(jt, 128)],
                            kT_sb[64:128, t, ts(ib, 512)],
                            start=True,
                            stop=True,
                        )
                        p = ppool.tile([128, 1024], bf16, tag="p")
                        nc.scalar.activation(p, s, Act.Exp)
                        if prev is not None:
                            pv(*prev)
                        prev = (jt, p)
                    pv(*prev)
                    nc.scalar.copy(outT_e[:, t, ts(ib, 512)], o_e)
                    nc.vector.tensor_copy(outT_o[:, t, ts(ib, 512)], o_o)

                y_tiles = {}
                mv_tiles = {}

                def partial(it, t):
                    # PE-transpose outT head-pair t for i-tile it into ydn,
                    # then divide by the softmax denominators right away
                    ydnp, fin, fsc, fps = fps_box["pools"]
                    if it not in ydn_tiles:
                        ydn_tiles[it] = ydnp.tile(
                            [128, 8, 65], f32, tag="ydn", name=f"ydn{it}"
                        )
                        y_tiles[it] = fin.tile(
                            [128, 512], f32, tag="y", name=f"y{it}", bufs=8
                        )
                    ydn = ydn_tiles[it]
                    y = y_tiles[it]
                    for so, srcT in ((0, outT_e), (1, outT_o)):
                        tp = fps.tile([128, 65], f32, tag="tp")
                        nc.tensor.transpose(
                            tp,
                            srcT[0:65, t, ts(it, 128)],
                            ident[0:65, 0:65],
                        )
                        nc.vector.tensor_copy(ydn[:, 2 * t + so, :], tp)
                    rc2 = fsc.tile([128, 2], f32, tag="rc2")
                    nc.vector.reciprocal(rc2, ydn[:, 2 * t : 2 * t + 2, 0])
                    for so in range(2):
                        k = 2 * t + so
                        nc.vector.tensor_scalar_mul(
                            y[:, ts(k, 64)],
                            ydn[:, k, 1:65],
                            rc2[:, so : so + 1],
                        )

                def closing_a(it, ve):
                    ydnp, fin, fsc, fps = fps_box["pools"]
                    ydn_tiles.pop(it)
                    y = y_tiles[it]
                    nc.vector.tensor_add(y, y, vinres[:, it, :])
                    st = fsc.tile([128, 6], f32, tag="fst")
                    nc.vector.bn_stats(st, y)
                    mv = fsc.tile([128, 2], f32, tag="fmv", bufs=8)
                    nc.vector.bn_aggr(mv, st)
                    mv_tiles[it] = mv
                    nc.vector.tensor_scalar_add(ve, mv[:, 1:2], EPS)

                def closing_b(it, rstd2):
                    y = y_tiles.pop(it)
                    mv = mv_tiles.pop(it)
                    nc.vector.tensor_scalar(
                        out=y,
                        in0=y,
                        scalar1=mv[:, 0:1],
                        scalar2=rstd2,
                        op0=Alu.subtract,
                        op1=Alu.mult,
                    )
                    if has_gamma:
                        nc.vector.tensor_mul(y, y, gammab)
                    if has_beta:
                        nc.gpsimd.tensor_add(y, y, betab)
                    nc.sync.dma_start(out[:, it, :], y)

                def closing(it):
                    ydnp, fin, fsc, fps = fps_box["pools"]
                    ve = fsc.tile([128, 1], f32, tag="fve")
                    closing_a(it, ve)
                    rstd2 = fsc.tile([128, 1], f32, tag="frs")
                    newton_rsqrt(rstd2, ve, 1, fsc)
                    closing_b(it, rstd2)

                def closing_batch(its):
                    ydnp, fin, fsc, fps = fps_box["pools"]
                    ve4 = fsc.tile([128, 4], f32, tag="fve4")
                    for j, it in enumerate(its):
                        closing_a(it, ve4[:, j : j + 1])
                    rstd4 = fsc.tile([128, 4], f32, tag="frs4")
                    newton_rsqrt(rstd4, ve4, 4, fsc)
                    for j, it in enumerate(its):
                        closing_b(it, rstd4[:, j : j + 1])

                # interleave tables: JIT projections + v-proj stream;
                # vinres rides later blocks' spare DVE slots
                def il_b00(jt):
                    if jt < 12:
                        vproj(jt + 4)
                    if jt in (1, 5, 9):
                        qproj(jt // 4 + 1, 0)
                    elif jt == 13:
                        qproj(0, 1)
                    elif jt == 14:
                        kproj(0, 1)
                    elif jt == 15:
                        vinres_unit(0)

                def il_b10(jt):
                    if jt in (1, 5, 9):
                        qproj((jt + 3) // 4, 1)
                    elif jt == 2:
                        qproj(0, 2)
                    elif jt == 6:
                        kproj(0, 2)
                    elif jt == 10:
                        qproj(0, 3)
                    elif jt == 12:
                        kproj(0, 3)
                    elif jt in (3, 7, 11, 13):
                        vinres_unit(1 + (jt - 3) // 4 + (1 if jt == 13 else 0))

                def il_b20(jt):
                    if jt in (1, 3, 5):
                        qproj((jt + 1) // 2, 2)
                    elif jt in (7, 9, 11):
                        qproj((jt - 5) // 2, 3)
                    elif jt >= 12:
                        kproj(1, jt - 12)
                    elif jt in (2, 4, 6):
                        vinres_unit(5 + (jt - 2) // 2)

                attn_block(0, 0, interleave=il_b00)
                attn_block(1, 0, interleave=il_b10)
                attn_block(2, 0, interleave=il_b20)
                # all projections emitted; free their SBUF/PSUM for the
                # finalize pools
                pp_pool.release()
                vs_pool.release()
                st_pool.release()
                w3_pool.release()
                sk_pool.release()
                qs_pool.release()
                wq_pool.release()
                fps_box["pools"] = (
                    tc.alloc_tile_pool(name="ydnp", bufs=4),
                    tc.alloc_tile_pool(name="fin", bufs=4),
                    tc.alloc_tile_pool(name="fsc", bufs=4),
                    tc.alloc_tile_pool(name="fps", bufs=2, space="PSUM"),
                )
                for t2 in range(3):
                    for it in range(4):
                        partial(it, t2)
                attn_block(3, 0)
                for it in range(4):
                    partial(it, 3)

                def make_il_ib1(t):
                    def il(jt):
                        if t > 0 and jt < 4:
                            partial(4 + jt, t - 1)
                        if jt == 6 and t > 0:
                            closing(t - 1)
                        elif jt == 10 and t == 3:
                            closing(3)

                    return il

                for t in range(DT):
                    attn_block(t, 1, interleave=make_il_ib1(t))
                for it in range(4, 8):
                    partial(it, 3)
                closing_batch(range(4, 8))
                for pool in reversed(fps_box["pools"]):
                    pool.release()
            ppool.release()
            ops.release()
            sps.release()

    nc.compile()
    return nc


def _to_tiles_T(x):
    # [N, 512] -> [128, 4, N] f16 : out[p, t, n] = x[n, 128*t + p]
    n = x.shape[0]
    return np.ascontiguousarray(
        x.T.reshape(ET, 128, n).transpose(1, 0, 2), dtype=np.float16
    )


def _w_tiles(w):
    # [512, 512] (e, d) -> [128, 4, 512] f16 : out[p, t, d] = w[128*t + p, d]
    return np.ascontiguousarray(
        w.reshape(ET, 128, D).transpose(1, 0, 2), dtype=np.float16
    )


def kernel(seq_k, seq_q, seq_v, W1, W2, W3, gamma, beta, _trace=False):
    seq_k = np.asarray(seq_k, dtype=np.float32)
    seq_q = np.asarray(seq_q, dtype=np.float32)
    seq_v = np.asarray(seq_v, dtype=np.float32)
    W1 = np.asarray(W1, dtype=np.float32)
    W2 = np.asarray(W2, dtype=np.float32)
    W3 = np.asarray(W3, dtype=np.float32)
    gamma = np.asarray(gamma, dtype=np.float32)
    beta = np.asarray(beta, dtype=np.float32)

    has_gamma = bool(np.any(gamma != 1.0))
    has_beta = bool(np.any(beta != 0.0))

    key = (has_gamma, has_beta)
    if key not in _cache:
        _cache[key] = _build(has_gamma, has_beta)
    nc = _cache[key]

    from concourse import bass_utils

    W3g = W3 * gamma[None, :]  # W3g[d, e] = W3[d, e] * gamma[e]
    g3v = np.ascontiguousarray((W3 @ gamma)[None, :], dtype=np.float32)
    c3vv = np.ascontiguousarray((W3 @ beta)[None, :], dtype=np.float32)
    w1t = _w_tiles(np.ascontiguousarray(W1.T))
    w2t = _w_tiles(np.ascontiguousarray(W2.T))
    w3t = _w_tiles(np.ascontiguousarray(W3g.T))
    gam = np.ascontiguousarray(gamma[None, :], dtype=np.float32)
    bet = np.ascontiguousarray(beta[None, :], dtype=np.float32)

    in_maps = []
    for c in range(NCORES):
        b, half = divmod(c, 2)
        lo, hi = half * IH, half * IH + IH
        perm = np.r_[lo:hi, 0:lo, hi:S]
        sq = seq_q[b][perm]
        svp = seq_v[b][perm]
        sk = seq_k[b, lo:hi]
        in_maps.append(
            {
                "sqT": _to_tiles_T(sq),
                "skT": _to_tiles_T(sk),
                "svT": _to_tiles_T(svp),
                "sv": np.ascontiguousarray(
                    svp.reshape(NT, 128, 512).transpose(1, 0, 2),
                    dtype=np.float16,
                ),
                "w1T": w1t,
                "w2T": w2t,
                "w3gT": w3t,
                "g3": g3v,
                "c3v": c3vv,
                "gamma": gam,
                "beta": bet,
            }
        )

    res = bass_utils.run_bass_kernel_spmd(
        nc, in_maps, core_ids=list(range(NCORES)), trace=_trace
    )
    global _last_run
    _last_run = res

    full = np.empty((B, S, D), dtype=np.float32)
    for c in range(NCORES):
        b, half = divmod(c, 2)
        o = res.results[c]["out"]  # [128, 8, 512]
        full[b, half * IH : (half + 1) * IH] = o.transpose(1, 0, 2).reshape(
            IH, D
        )
    return full


_last_run = None


# revision 23
# speedup vs baseline: 1.0449x; 1.0067x over previous
"""TRN2 Bass kernel for nn_MultiHeadAttention (B=4, S=2048, D=512, H=8).

Computation (per reference):
  v_in = LN(seq_v) ; q = seq_q@W1.T ; k = seq_k@W2.T ; v = v_in@W3.T
  scores[b,h,i,j] = k_i . q_j ; attn = softmax_j(scores) ; out = attn @ v
  out = LN(out + v_in)

Sharding (zero-communication): core c -> (batch b=c//2, i-half=c%2).
Each core computes all 8 heads for its 1024 output rows (the "i" index,
which indexes K rows), needing full q/v (all j) for its batch and the
i-half slice of k. The j axis is permuted host-side (own half first) so
one SPMD program serves all cores; softmax over j is permutation
invariant and the residual rows are j-tiles 0..7 by construction.

v2 restructure (vs the phase-serial baseline):
  - fully software-pipelined emission: LN stats, q/k/v projections and
    the attention blocks are interleaved so the first exp fires ~11us in
    (was ~75us); DMAs are spread over the sync/scalar/gpsimd queues in
    consumption order
  - ScalarE stream is pure exp (projection PSUM->SBUF copies moved to
    DVE; all rsqrt via DVE Newton iteration -> single ACT table load)
  - vaug (v values + softmax-ones column, ones first) and p (exp
    output) in bf16: halves SBUF, FWL-eligible PV weight loads; all
    accumulation stays f32
  - finalize split into per-(it,t) transpose partials (overlap later
    attention blocks) and per-it closings; softmax division via one
    reciprocal + 8 per-head tensor_scalar_muls
  - PE warmed with dummy matmuls during the DMA lead-in (HAM K=8/8)
"""

import numpy as np

B, S, D, H = 4, 2048, 512, 8
HD = D // H  # 64
EPS = 1e-5
NCORES = 8
IH = S // 2          # 1024 output rows per core
NT = S // 128        # 16 j token-tiles
ITILES = IH // 128   # 8 i-tiles
DT = D // 128        # 4 d-tiles (head pairs)
ET = D // 128        # 4 e-tiles (contraction)
NIB = IH // 512      # 2 i-blocks

_cache = {}


def _build(has_gamma: bool, has_beta: bool):
    import concourse.bacc as bacc
    import concourse.mybir as mybir
    import concourse.tile as tile
    from concourse.masks import make_identity

    f32 = mybir.dt.float32
    f32r = mybir.dt.float32r
    f16 = mybir.dt.float16
    bf16 = mybir.dt.bfloat16
    i32 = mybir.dt.int32
    Alu = mybir.AluOpType
    Act = mybir.ActivationFunctionType

    nc = bacc.Bacc(None, target_bir_lowering=False)

    sqT = nc.dram_tensor("sqT", [128, ET, S], f16, kind="ExternalInput")
    skT = nc.dram_tensor("skT", [128, ET, IH], f16, kind="ExternalInput")
    svT = nc.dram_tensor("svT", [128, ET, S], f16, kind="ExternalInput")
    sv = nc.dram_tensor("sv", [128, NT, 512], f16, kind="ExternalInput")
    w1T = nc.dram_tensor("w1T", [128, ET, D], f16, kind="ExternalInput")
    w2T = nc.dram_tensor("w2T", [128, ET, D], f16, kind="ExternalInput")
    w3gT = nc.dram_tensor("w3gT", [128, ET, D], f16, kind="ExternalInput")
    g3 = nc.dram_tensor("g3", [1, D], f32, kind="ExternalInput")
    c3v = nc.dram_tensor("c3v", [1, D], f32, kind="ExternalInput")
    gamma = nc.dram_tensor("gamma", [1, D], f32, kind="ExternalInput")
    beta = nc.dram_tensor("beta", [1, D], f32, kind="ExternalInput")
    out = nc.dram_tensor("out", [128, ITILES, D], f32, kind="ExternalOutput")

    def bcast(dram_ap):
        import concourse.bass as bass

        return bass.AP(
            tensor=dram_ap.tensor,
            offset=dram_ap.offset,
            ap=[[0, 128], [1, D]],
        )

    ts = lambda i, sz: slice(i * sz, (i + 1) * sz)

    with tile.TileContext(nc) as tc:
        with (
            tc.tile_pool(name="const", bufs=1) as const,
            tc.tile_pool(name="persist", bufs=1) as persist,
        ):
            # long-lived attention pools first (pool releases must be
            # LIFO, and these outlive the projection-phase pools)
            sps = tc.alloc_tile_pool(name="sps", bufs=2, space="PSUM")
            ops = tc.alloc_tile_pool(name="ops", bufs=1, space="PSUM")
            ppool = tc.alloc_tile_pool(name="ppool", bufs=3)

            # ---- DMA, queue sync (q/k path, in consumption order) ----
            wq_pool = tc.alloc_tile_pool(name="wqk", bufs=1)
            qs_pool = tc.alloc_tile_pool(name="qs", bufs=4)
            sk_pool = tc.alloc_tile_pool(name="sk", bufs=2)
            w1_sb = wq_pool.tile([128, ET, D], f16, tag="w1")
            nc.sync.dma_start(w1_sb, w1T[:])
            sqc = {}
            skc = {}
            sqc[0] = qs_pool.tile([128, ET, 512], f16, tag="sqc", name="sqc0")
            nc.sync.dma_start(sqc[0], sqT[:, :, 0:512])
            skc[0] = sk_pool.tile([128, ET, 512], f16, tag="skc", name="skc0")
            nc.sync.dma_start(skc[0], skT[:, :, 0:512])
            # (sqc1..3, skc1 issued below, after the first compute units)
            _anchors = {}
            _held = []

            def held(bi, key="v"):
                # per-queue issue order is the priority mechanism; explicit
                # completion-sem holds measured ~10us release latency (worse)
                return bi

            # ---- DMA, queue scalar (w2 only; before any exp) ----
            w2_sb = wq_pool.tile([128, ET, D], f16, tag="w2")
            nc.scalar.dma_start(w2_sb, w2T[:])

            # ---- DMA, queues scalar/gpsimd (v path + consts) ----
            g3b = const.tile([128, D], f32, tag="g3b")
            _g3b_dma = nc.gpsimd.dma_start(g3b, bcast(g3[:]))
            if has_gamma:
                gammab = const.tile([128, D], f32, tag="gammab")
                nc.gpsimd.dma_start(gammab, bcast(gamma[:]))
            if has_beta:
                betab = const.tile([128, D], f32, tag="betab")
                nc.gpsimd.dma_start(betab, bcast(beta[:]))
                c3b = const.tile([128, D], f32, tag="c3b")
                nc.gpsimd.dma_start(c3b, bcast(c3v[:]))
            w3_pool = tc.alloc_tile_pool(name="w3p", bufs=1)
            w3_sb = w3_pool.tile([128, ET, D], f16, tag="w3")
            nc.scalar.dma_start(w3_sb, w3gT[:])



            # persistent intermediates
            qT_sb = persist.tile([128, DT, S], f16, tag="qT")
            kT_sb = persist.tile([128, DT, IH], f16, tag="kT")
            vaug = persist.tile([128, NT, H, 65], bf16, tag="vaug")
            outT_e = persist.tile([65, DT, IH], f32, tag="outTe")
            outT_o = persist.tile([65, DT, IH], f32, tag="outTo")
            vinres = persist.tile([128, ITILES, 512], f16, tag="vinres")
            mu16 = persist.tile([128, NT], f32, tag="mu16")
            ve16 = persist.tile([128, NT], f32, tag="ve16")
            rstd16 = persist.tile([128, NT], f32, tag="rstd16")
            mr16 = persist.tile([128, NT], f32, tag="mr16")

            # sv own-half lands directly in vinres (LN applied in-place
            # later); other-half tiles stream through the stat pool
            nc.gpsimd.dma_start(vinres[:, 0:4, :], sv[:, 0:4, :])
            nc.gpsimd.dma_start(vinres[:, 4:8, :], sv[:, 4:8, :])
            svc = {}
            st_pool = tc.alloc_tile_pool(name="stat", bufs=2)
            vs_pool = tc.alloc_tile_pool(name="vs", bufs=3)
            for c in range(8):
                svc[c] = vs_pool.tile(
                    [128, ET, 256], f16, tag="svc", name=f"svc{c}"
                )
                eng = nc.scalar if c < 4 else nc.gpsimd
                eng.dma_start(svc[c], svT[:, :, ts(c, 256)])
            # LN-stats input for the other j-half
            xoh = {}
            for c in range(4):
                xoh[c] = st_pool.tile(
                    [128, 2, 512], f16, tag="x", name=f"xoh{c}", bufs=2
                )
                nc.gpsimd.dma_start(xoh[c], sv[:, ts(c + 4, 2), :])

            onesc = const.tile([128, NT * H], bf16, tag="onesc")
            nc.vector.memset(onesc, 1.0)
            nc.vector.tensor_copy(
                vaug[:, :, :, 0],
                onesc.rearrange("p (a b) -> p a b", a=NT),
            )
            ident = const.tile([128, 128], f32, tag="ident")
            make_identity(nc, ident)

            pp_pool = tc.alloc_tile_pool(name="pp", bufs=2, space="PSUM")

            # ---- helper units ----
            def newton_rsqrt(dst, src, n, tmp_pool):
                # dst = 1/sqrt(src) elementwise on [128, n] (DVE only)
                nc.vector.tensor_scalar(
                    out=dst.bitcast(i32),
                    in0=src.bitcast(i32),
                    scalar1=1,
                    scalar2=None,
                    op0=Alu.logical_shift_right,
                )
                nc.vector.tensor_scalar(
                    out=dst.bitcast(i32),
                    in0=dst.bitcast(i32),
                    scalar1=-1,
                    scalar2=0x5F3759DF,
                    op0=Alu.mult,
                    op1=Alu.add,
                )
                tmp1 = tmp_pool.tile([128, NT], f32, tag="nwt")
                t1 = tmp1[:, 0:n]
                for _ in range(2):
                    nc.vector.tensor_mul(t1, dst, dst)
                    nc.vector.tensor_mul(t1, t1, src)
                    nc.vector.tensor_scalar(
                        out=t1,
                        in0=t1,
                        scalar1=-0.5,
                        scalar2=1.5,
                        op0=Alu.mult,
                        op1=Alu.add,
                    )
                    nc.vector.tensor_mul(dst, dst, t1)

            def stats_unit(jt, x):
                # LN stats of token tile jt from x = [128, 512]
                st = st_pool.tile([128, 6], f32, tag="st")
                nc.vector.bn_stats(st, x)
                mv = st_pool.tile([128, 2], f32, tag="mv")
                nc.vector.bn_aggr(mv, st)
                nc.vector.tensor_copy(mu16[:, jt : jt + 1], mv[:, 0:1])
                nc.vector.tensor_scalar_add(
                    ve16[:, jt : jt + 1], mv[:, 1:2], EPS
                )

            def rstd_batch(lo, hi):
                newton_rsqrt(
                    rstd16[:, lo:hi], ve16[:, lo:hi], hi - lo, st_pool
                )
                nc.vector.tensor_mul(
                    mr16[:, lo:hi], mu16[:, lo:hi], rstd16[:, lo:hi]
                )

            def qproj(jc, t):
                ps = pp_pool.tile([128, 512], f32, tag="pp")
                for e in range(ET):
                    nc.tensor.matmul(
                        ps,
                        w1_sb[:, e, ts(t, 128)],
                        sqc[jc][:, e, :],
                        start=(e == 0),
                        stop=(e == ET - 1),
                    )
                nc.vector.tensor_copy(qT_sb[:, t, ts(jc, 512)], ps)

            def kproj(ic, t):
                ps = pp_pool.tile([128, 512], f32, tag="pp")
                for e in range(ET):
                    nc.tensor.matmul(
                        ps,
                        w2_sb[:, e, ts(t, 128)],
                        skc[ic][:, e, :],
                        start=(e == 0),
                        stop=(e == ET - 1),
                    )
                nc.vector.tensor_copy(kT_sb[:, t, ts(ic, 512)], ps)

            def vproj(jt):
                # v row jt: vaug[:, jt, h, 1:65] = rstd*(sv@W3g.T) - mr*g3 (+c3)
                ps = pp_pool.tile([128, 512], f32, tag="pp")
                c, r = divmod(jt, 2)
                for e in range(ET):
                    nc.tensor.matmul(
                        ps,
                        svc[c][:, e, ts(r, 128)],
                        w3_sb[:, e, :],
                        start=(e == 0),
                        stop=(e == ET - 1),
                    )
                tA = vs_pool.tile([128, 512], bf16, tag="tA", bufs=2)
                nc.vector.tensor_scalar_mul(tA, g3b, mr16[:, jt : jt + 1])
                tB = vs_pool.tile([128, 512], bf16, tag="tB", bufs=2)
                nc.vector.tensor_scalar_mul(tB, ps, rstd16[:, jt : jt + 1])
                vdst = vaug[:, jt, :, 1:65]
                nc.vector.tensor_tensor(
                    out=vdst,
                    in0=tB.rearrange("p (h d) -> p h d", h=H),
                    in1=tA.rearrange("p (h d) -> p h d", h=H),
                    op=Alu.subtract,
                )
                if has_beta:
                    nc.gpsimd.tensor_add(
                        vdst,
                        vdst,
                        c3b.rearrange("p (h d) -> p h d", h=H),
                    )

            def vinres_unit(jt):
                # in-place LN of own-half sv tile (residual v_in)
                nc.vector.tensor_scalar(
                    out=vinres[:, jt, :],
                    in0=vinres[:, jt, :],
                    scalar1=mu16[:, jt : jt + 1],
                    scalar2=rstd16[:, jt : jt + 1],
                    op0=Alu.subtract,
                    op1=Alu.mult,
                )
                if has_gamma:
                    nc.vector.tensor_mul(
                        vinres[:, jt, :], vinres[:, jt, :], gammab
                    )
                if has_beta:
                    nc.gpsimd.tensor_add(
                        vinres[:, jt, :], vinres[:, jt, :], betab
                    )

            # ---- phase A: minimal prefix during the DMA lead-in ----
            for jt in range(4):
                stats_unit(jt, vinres[:, jt, :])
            qproj(0, 0)
            kproj(0, 0)
            rstd_batch(0, 4)
            # remaining q-path DMAs (sync queue, in need order)
            for jc in range(1, 4):
                sqc[jc] = qs_pool.tile(
                    [128, ET, 512], f16, tag="sqc", name=f"sqc{jc}"
                )
                nc.sync.dma_start(sqc[jc], sqT[:, :, ts(jc, 512)])
            skc[1] = sk_pool.tile([128, ET, 512], f16, tag="skc", name="skc1")
            nc.sync.dma_start(skc[1], skT[:, :, 512:1024])
            for jt in range(4, 8):
                stats_unit(jt, vinres[:, jt, :])
            rstd_batch(4, 8)
            for jt in range(4):
                vproj(jt)
            for jt in range(8, 16):
                stats_unit(jt, xoh[(jt - 8) // 2][:, (jt - 8) % 2, :])
            rstd_batch(8, 16)

            # ---- attention + overlapped projections/finalize ----
            ydn_tiles = {}
            fps_box = {}

            if True:

                def attn_block(t, ib, interleave=None):
                    o_e = ops.tile([65, 512], f32, tag="oe")
                    o_o = ops.tile([65, 512], f32, tag="oo")

                    def pv(jt, p):
                        nc.tensor.matmul(
                            o_e,
                            vaug[:, jt, 2 * t, :],
                            p[:, 0:512],
                            start=(jt == 0),
                            stop=(jt == NT - 1),
                        )
                        nc.tensor.matmul(
                            o_o,
                            vaug[:, jt, 2 * t + 1, :],
                            p[:, 512:1024],
                            start=(jt == 0),
                            stop=(jt == NT - 1),
                        )

                    prev = None
                    for jt in range(NT):
                        if interleave is not None:
                            interleave(jt)
                        s = sps.tile([128, 1024], f32, tag="s")
                        nc.tensor.matmul(
                            s[:, 0:512],
                            qT_sb[0:64, t, ts(jt, 128)],
                            kT_sb[0:64, t, ts(ib, 512)],
                            start=True,
                            stop=True,
                        )
                        nc.tensor.matmul(
                            s[:, 512:1024],
                            qT_sb[64:128, t, ts(jt, 128)],
                            kT_sb[64:128, t, ts(ib, 512)],
                            start=True,
                            stop=True,
                        )
                        p = ppool.tile([128, 1024], bf16, tag="p")
                        nc.scalar.activation(p, s, Act.Exp)
                        if prev is not None:
                            pv(*prev)
                        prev = (jt, p)
                    pv(*prev)
                    nc.scalar.copy(outT_e[:, t, ts(ib, 512)], o_e)
                    nc.vector.tensor_copy(outT_o[:, t, ts(ib, 512)], o_o)

                y_tiles = {}
                mv_tiles = {}

                def partial(it, t):
                    # PE-transpose outT head-pair t for i-tile it into ydn,
                    # then divide by the softmax denominators right away
                    ydnp, fin, fsc, fps = fps_box["pools"]
                    if it not in ydn_tiles:
                        ydn_tiles[it] = ydnp.tile(
                            [128, 8, 65], f32, tag="ydn", name=f"ydn{it}"
                        )
                        y_tiles[it] = fin.tile(
                            [128, 512], f32, tag="y", name=f"y{it}", bufs=8
                        )
                    ydn = ydn_tiles[it]
                    y = y_tiles[it]
                    for so, srcT in ((0, outT_e), (1, outT_o)):
                        tp = fps.tile([128, 65], f32, tag="tp")
                        nc.tensor.transpose(
                            tp,
                            srcT[0:65, t, ts(it, 128)],
                            ident[0:65, 0:65],
                        )
                        nc.vector.tensor_copy(ydn[:, 2 * t + so, :], tp)
                    rc2 = fsc.tile([128, 2], f32, tag="rc2")
                    nc.vector.reciprocal(rc2, ydn[:, 2 * t : 2 * t + 2, 0])
                    for so in range(2):
                        k = 2 * t + so
                        nc.vector.tensor_scalar_mul(
                            y[:, ts(k, 64)],
                            ydn[:, k, 1:65],
                            rc2[:, so : so + 1],
                        )

                def closing_a(it, ve):
                    ydnp, fin, fsc, fps = fps_box["pools"]
                    ydn_tiles.pop(it)
                    y = y_tiles[it]
                    nc.vector.tensor_add(y, y, vinres[:, it, :])
                    st = fsc.tile([128, 6], f32, tag="fst")
                    nc.vector.bn_stats(st, y)
                    mv = fsc.tile([128, 2], f32, tag="fmv", bufs=8)
                    nc.vector.bn_aggr(mv, st)
                    mv_tiles[it] = mv
                    nc.vector.tensor_scalar_add(ve, mv[:, 1:2], EPS)

                def closing_b(it, rstd2):
                    y = y_tiles.pop(it)
                    mv = mv_tiles.pop(it)
                    nc.vector.tensor_scalar(
                        out=y,
                        in0=y,
                        scalar1=mv[:, 0:1],
                        scalar2=rstd2,
                        op0=Alu.subtract,
                        op1=Alu.mult,
                    )
                    if has_gamma:
                        nc.vector.tensor_mul(y, y, gammab)
                    if has_beta:
                        nc.gpsimd.tensor_add(y, y, betab)
                    nc.sync.dma_start(out[:, it, :], y)

                def closing(it):
                    ydnp, fin, fsc, fps = fps_box["pools"]
                    ve = fsc.tile([128, 1], f32, tag="fve")
                    closing_a(it, ve)
                    rstd2 = fsc.tile([128, 1], f32, tag="frs")
                    newton_rsqrt(rstd2, ve, 1, fsc)
                    closing_b(it, rstd2)

                def closing_batch(its):
                    ydnp, fin, fsc, fps = fps_box["pools"]
                    ve4 = fsc.tile([128, 4], f32, tag="fve4")
                    for j, it in enumerate(its):
                        closing_a(it, ve4[:, j : j + 1])
                    rstd4 = fsc.tile([128, 4], f32, tag="frs4")
                    newton_rsqrt(rstd4, ve4, 4, fsc)
                    for j, it in enumerate(its):
                        closing_b(it, rstd4[:, j : j + 1])

                # interleave tables: JIT projections + v-proj stream;
                # vinres rides later blocks' spare DVE slots
                def il_b00(jt):
                    if jt < 12:
                        vproj(jt + 4)
                    if jt in (1, 5, 9):
                        qproj(jt // 4 + 1, 0)
                    elif jt == 13:
                        qproj(0, 1)
                    elif jt == 14:
                        kproj(0, 1)
                    elif jt == 15:
                        vinres_unit(0)

                def il_b10(jt):
                    if jt in (1, 5, 9):
                        qproj((jt + 3) // 4, 1)
                    elif jt == 2:
                        qproj(0, 2)
                    elif jt == 6:
                        kproj(0, 2)
                    elif jt == 10:
                        qproj(0, 3)
                    elif jt == 12:
                        kproj(0, 3)
                    elif jt in (3, 7, 11, 13):
                        vinres_unit(1 + (jt - 3) // 4 + (1 if jt == 13 else 0))

                def il_b20(jt):
                    if jt in (1, 3, 5):
                        qproj((jt + 1) // 2, 2)
                    elif jt in (7, 9, 11):
                        qproj((jt - 5) // 2, 3)
                    elif jt >= 12:
                        kproj(1, jt - 12)
                    elif jt in (2, 4, 6):
                        vinres_unit(5 + (jt - 2) // 2)

                attn_block(0, 0, interleave=il_b00)
                attn_block(1, 0, interleave=il_b10)
                attn_block(2, 0, interleave=il_b20)
                # all projections emitted; free their SBUF/PSUM for the
                # finalize pools
                pp_pool.release()
                vs_pool.release()
                st_pool.release()
                w3_pool.release()
                sk_pool.release()
                qs_pool.release()
                wq_pool.release()
                fps_box["pools"] = (
                    tc.alloc_tile_pool(name="ydnp", bufs=4),
                    tc.alloc_tile_pool(name="fin", bufs=4),
                    tc.alloc_tile_pool(name="fsc", bufs=4),
                    tc.alloc_tile_pool(name="fps", bufs=2, space="PSUM"),
                )
                for t2 in range(3):
                    for it in range(4):
                        partial(it, t2)
                attn_block(3, 0)
                for it in range(4):
                    partial(it, 3)

                def make_il_ib1(t):
                    def il(jt):
                        if t > 0 and jt < 4:
                            partial(4 + jt, t - 1)
                        if jt == 6 and t > 0:
                            closing(t - 1)
                        elif jt == 10 and t == 3:
                            closing(3)

                    return il

                for t in range(DT):
                    attn_block(t, 1, interleave=make_il_ib1(t))
                for it in range(4, 8):
                    partial(it, 3)
                closing_batch(range(4, 8))
                for pool in reversed(fps_box["pools"]):
                    pool.release()
            ppool.release()
            ops.release()
            sps.release()

    nc.compile()
    return nc


def _to_tiles_T(x):
    # [N, 512] -> [128, 4, N] f16 : out[p, t, n] = x[n, 128*t + p]
    n = x.shape[0]
    return np.ascontiguousarray(
        x.T.reshape(ET, 128, n).transpose(1, 0, 2), dtype=np.float16
    )


def _w_tiles(w):
    # [512, 512] (e, d) -> [128, 4, 512] f16 : out[p, t, d] = w[128*t + p, d]
    return np.ascontiguousarray(
        w.reshape(ET, 128, D).transpose(1, 0, 2), dtype=np.float16
    )


def kernel(seq_k, seq_q, seq_v, W1, W2, W3, gamma, beta, _trace=False):
    seq_k = np.asarray(seq_k, dtype=np.float32)
    seq_q = np.asarray(seq_q, dtype=np.float32)
    seq_v = np.asarray(seq_v, dtype=np.float32)
    W1 = np.asarray(W1, dtype=np.float32)
    W2 = np.asarray(W2, dtype=np.float32)
    W3 = np.asarray(W3, dtype=np.float32)
    gamma = np.asarray(gamma, dtype=np.float32)
    beta = np.asarray(beta, dtype=np.float32)

    has_gamma = bool(np.any(gamma != 1.0))
    has_beta = bool(np.any(beta != 0.0))

    key = (has_gamma, has_beta)
    if key not in _cache:
        _cache[key] = _build(has_gamma, has_beta)
    nc = _cache[key]

    from concourse import bass_utils

    W3g = W3 * gamma[None, :]  # W3g[d, e] = W3[d, e] * gamma[e]
    g3v = np.ascontiguousarray((W3 @ gamma)[None, :], dtype=np.float32)
    c3vv = np.ascontiguousarray((W3 @ beta)[None, :], dtype=np.float32)
    w1t = _w_tiles(np.ascontiguousarray(W1.T))
    w2t = _w_tiles(np.ascontiguousarray(W2.T))
    w3t = _w_tiles(np.ascontiguousarray(W3g.T))
    gam = np.ascontiguousarray(gamma[None, :], dtype=np.float32)
    bet = np.ascontiguousarray(beta[None, :], dtype=np.float32)

    in_maps = []
    for c in range(NCORES):
        b, half = divmod(c, 2)
        lo, hi = half * IH, half * IH + IH
        perm = np.r_[lo:hi, 0:lo, hi:S]
        sq = seq_q[b][perm]
        svp = seq_v[b][perm]
        sk = seq_k[b, lo:hi]
        in_maps.append(
            {
                "sqT": _to_tiles_T(sq),
                "skT": _to_tiles_T(sk),
                "svT": _to_tiles_T(svp),
                "sv": np.ascontiguousarray(
                    svp.reshape(NT, 128, 512).transpose(1, 0, 2),
                    dtype=np.float16,
                ),
                "w1T": w1t,
                "w2T": w2t,
                "w3gT": w3t,
                "g3": g3v,
                "c3v": c3vv,
                "gamma": gam,
                "beta": bet,
            }
        )

    res = bass_utils.run_bass_kernel_spmd(
        nc, in_maps, core_ids=list(range(NCORES)), trace=_trace
    )
    global _last_run
    _last_run = res

    full = np.empty((B, S, D), dtype=np.float32)
    for c in range(NCORES):
        b, half = divmod(c, 2)
        o = res.results[c]["out"]  # [128, 8, 512]
        full[b, half * IH : (half + 1) * IH] = o.transpose(1, 0, 2).reshape(
            IH, D
        )
    return full


_last_run = None
